# revision 8
# baseline (speedup 1.0000x reference)
"""v6b: dual-ring variant. Sync ring: all yhat halves + all small-tile y
halves. Scalar ring: big-tile y halves only, with dma_starts emitted one
stagger-slot AHEAD of the activations in the ACT program so an activation
waiting on data never delays a DMA issue. Compute split as v5/v6:
DVE = subtract only, ACT = activation(Abs, accum_out)."""

import numpy as np

import concourse.bacc as bacc
import concourse.mybir as mybir
import concourse.tile as tile
from concourse.bass_utils import run_bass_kernel_spmd

N_CORES = 8
FULL_SHAPE = (64, 128, 4096)
TOTAL_ELEMS = FULL_SHAPE[0] * FULL_SHAPE[1] * FULL_SHAPE[2]

P = 128
ELEMS_PER_CORE = TOTAL_ELEMS // N_CORES
F_TOTAL = ELEMS_PER_CORE // P

F_BIG = [4096] * 7
F_SMALL = [2048, 1024, 512, 256, 256]
F_TILES = F_BIG + F_SMALL
assert sum(F_TILES) == F_TOTAL
N_TILES = len(F_TILES)
N_BIG = len(F_BIG)

_nc_cache = []


def _build_nc():
    nc = bacc.Bacc("TRN2", target_bir_lowering=False, debug=False)
    yh = nc.declare_dram_parameter("yh", [P, F_TOTAL], mybir.dt.float32, isOutput=False)
    yy = nc.declare_dram_parameter("yy", [P, F_TOTAL], mybir.dt.float32, isOutput=False)
    out = nc.declare_dram_parameter("out", [P, N_TILES], mybir.dt.float32, isOutput=True)

    offs = []
    o = 0
    for f in F_TILES:
        offs.append(o)
        o += f

    with tile.TileContext(nc) as tc:
        with (
            tc.tile_pool(name="ina", bufs=3) as a_pool,
            tc.tile_pool(name="inb", bufs=3) as b_pool,
            tc.tile_pool(name="diff", bufs=2) as diff_pool,
            tc.tile_pool(name="small", bufs=1) as small_pool,
            tc.tile_pool(name="acc", bufs=1) as acc_pool,
        ):
            acc = acc_pool.tile([P, N_TILES], mybir.dt.float32)
            sc = acc_pool.tile([P, 4096], mybir.dt.float32, tag="scratch")
            ats, bts, ds = [], [], []
            for i, f in enumerate(F_TILES):
                if f == 4096:
                    ats.append(a_pool.tile([P, f], mybir.dt.float32, tag="a", name=f"a{i}"))
                    bts.append(b_pool.tile([P, f], mybir.dt.float32, tag="b", name=f"b{i}"))
                else:
                    ats.append(small_pool.tile([P, f], mybir.dt.float32, tag=f"a{i}", name=f"a{i}"))
                    bts.append(small_pool.tile([P, f], mybir.dt.float32, tag=f"b{i}", name=f"b{i}"))
                ds.append(diff_pool.tile([P, f], mybir.dt.float32, tag="d", name=f"d{i}"))

            def load(i):
                f = F_TILES[i]
                nc.sync.dma_start(ats[i][:], yh[:, offs[i] : offs[i] + f])
                eng = nc.scalar if i < N_BIG else nc.sync
                eng.dma_start(bts[i][:], yy[:, offs[i] : offs[i] + f])

            def compute(i):
                f = F_TILES[i]
                nc.vector.tensor_sub(ds[i][:], ats[i][:], bts[i][:])
                nc.scalar.activation(
                    sc[:, 0:f],
                    ds[i][:],
                    mybir.ActivationFunctionType.Abs,
                    accum_out=acc[:, i : i + 1],
                )

            # Stagger: loads run 3 tiles ahead of compute in program order so
            # the ACT-ring dma_starts precede the activations that could
            # block them.
            LEAD = 3
            for i in range(LEAD):
                load(i)
            for i in range(N_TILES):
                if i + LEAD < N_TILES:
                    load(i + LEAD)
                compute(i)
            nc.scalar.dma_start(out[:], acc[:])
    nc.compile()
    return nc


def _get_nc():
    if not _nc_cache:
        _nc_cache.append(_build_nc())
    return _nc_cache[0]


def _shard_inputs(yhat: np.ndarray, y: np.ndarray) -> list[dict[str, np.ndarray]]:
    yh = np.ascontiguousarray(yhat, dtype=np.float32).reshape(N_CORES, P, F_TOTAL)
    yy = np.ascontiguousarray(y, dtype=np.float32).reshape(N_CORES, P, F_TOTAL)
    return [{"yh": yh[c], "yy": yy[c]} for c in range(N_CORES)]


def kernel(yhat: np.ndarray, y: np.ndarray) -> np.ndarray:
    nc = _get_nc()
    in_maps = _shard_inputs(yhat, y)
    res = run_bass_kernel_spmd(nc, in_maps, list(range(N_CORES)))
    total = np.float64(0.0)
    for r in res.results:
        total += r["out"].astype(np.float64).sum()
    return np.asarray(total / TOTAL_ELEMS, dtype=np.float32)
